# revision 32
# baseline (speedup 1.0000x reference)
"""Trainium2 Bass kernel for nn_BaseNet_72533407694985.

Computes, per batch b:
  p = pts @ rot_b + trans_b            (pts = pointclouds[b,:, :3])
  valid = (p_x^2+p_y^2 < 1) & (p_z < 1) & (sum(normals) != 0)
  out[b] = stable-compact rows of pointclouds[b] where valid, zero tail.

Strategy (4 batches per core on 8 cores, unified 128-partition tiles):
  - Partition p = (batch b = p//32, slab s = p%32); each partition owns the
    contiguous 4096-point slab [s*4096, (s+1)*4096) of its batch, so every
    op covers all 4 batches at once with per-partition transform scalars.
  - Per column-chunk: ACT deinterleaves the point data (cp01 = (x,y) pairs,
    zt = z; stride-24B reads halve DVE throughput, so the copies are paid
    once on the otherwise-idle ACT) and computes b_e = z*r2e + t_e
    (scale/bias); DVE runs the six exact-chain fmas (stt) plus two fused
    custom DVE ops (registered at build time in dve_ops.OPS):
      CMP_Z  : m0 = (x*r02 + u2) < 1         (mult,add,is_lt — 3 stages)
      CMP_XY : m1 = (p0*p0 + p1*p1) < 1      (sq,sq,add,is_lt — 4 stages)
    writing the two geometric mask bytes directly (u8 0/1). Mask-out DMAs
    ride the otherwise-idle SP engine so their waits never stall compute.
  - Host applies the (bit-exact in numpy f32) padded-row check nsum != 0,
    ANDs the mask bytes, and does the stable compaction — same split the
    previous version of this kernel family used.
  - Arithmetic association kept bit-identical to the reference chain
    (z*r2e+t_e via ACT scale/bias, += y*r1e, += x*r0e via stt; squares,
    sum and compares as exact f32 mult/add/is_lt on the DVE ALUs).
"""

import numpy as np

B = 32
N = 131072
C = 6
P = 128
NCORES = 8
BPC = B // NCORES   # batches per core
SLABS = 32          # slabs per batch; BPC*SLABS = 128 partitions
W = N // SLABS      # 4096 points per partition

# (col0, width) chunks over the 4096-point slab: small chunks up front for
# a fast ramp, then a monotone taper [1024, 896, 768, 640]. The taper has
# the same chunk/op count as [1024,1024,1024,256] (so no cost when the run
# is DVE-bound) but shortens the serial delivery->copy->compute trail of
# the final grains when HBM bandwidth is degraded and T is delivery-bound
# (measured ~0.6-1.7us across taper steps at 38-41us stream spans).
CHUNKS = [(0, 256), (256, 512), (768, 768), (1536, 1024), (2560, 896), (3456, 640)]
# compute chunks: (col0, width, [grain indices]) — 1:1 with the DMA grains
# (wider fused compute chunks measured slower: they defer compute behind the
# later grain's delivery, which outweighs the saved per-op fixed cost)
CCHUNKS = [(c0, cw, [i]) for i, (c0, cw) in enumerate(CHUNKS)]

_CACHE = {}
SPILL_WAITS = True


def _split_excess_waits(nc):
    """Walrus codegen caps sync waits at 1 per instruction (2 for
    EventSemaphore). Spill extra waits into sem-only EventSemaphore nops
    inserted just before the overloaded instruction on the same engine."""
    from concourse import mybir

    n_spilled = 0
    for f in nc.m.functions:
        for blk in f.blocks:
            out = []
            changed = False
            for ins in blk.instructions:
                si = ins.sync_info
                cap = 2 if isinstance(ins, mybir.InstEventSemaphore) else 1
                if si is not None and len(si.on_wait) > cap:
                    waits = list(si.on_wait)
                    keep, spill = waits[:cap], waits[cap:]
                    k = 0
                    while spill:
                        chunk, spill = spill[:2], spill[2:]
                        out.append(
                            mybir.InstEventSemaphore(
                                name=f"{ins.name}_w{k}",
                                engine=ins.engine,
                                ins=[],
                                outs=[],
                                sync_info=mybir.SyncInfo(
                                    on_wait=chunk, on_update=[]
                                ),
                            )
                        )
                        k += 1
                        n_spilled += 1
                    si.on_wait = keep
                    changed = True
                out.append(ins)
            if changed:
                blk.instructions = out
    return n_spilled


def _register_ops():
    """Register the two fused mask ops in the custom-DVE registry (the
    documented extension point: append a DveOp to dve_ops.OPS). Idempotent."""
    import concourse.dve_ops as dvo
    from concourse.dve_spec import Spec, Src0, Src1, C0, One, sq, lower
    from concourse.dve_spec import _has_src1
    from concourse.dve_uop import DveOpSpec

    def reg(name, spec):
        for o in dvo.OPS:
            if o.name == name:
                return o
        row = dvo._CUSTOM_DVE_ROW_BASE + len(dvo.OPS)
        assert row < 0x20, "custom DVE opcode rows exhausted"
        dvo._SUB_OPCODE_FOR_NAME[name] = row
        sha = DveOpSpec(
            name=name,
            opcode=row,
            uops=lower(spec, ver="v3"),
            rd1_en=_has_src1(spec),
        ).sha("v3")
        op = dvo.DveOp(name, spec, subdim=False, uops_sha={"v3": sha})
        dvo.OPS.append(op)
        dvo.CUSTOM_DVE_SPECS[name] = spec
        return op

    one = np.float32(1.0)

    def _ref_cmp_xy(in0, in1, s0, s1, imm2):
        a = np.asarray(in0, np.float32)
        b = np.asarray(in1, np.float32)
        return ((a * a + b * b) < one).astype(np.float32)

    def _ref_cmp_z(in0, in1, s0, s1, imm2):
        a = np.asarray(in0, np.float32)
        b = np.asarray(in1, np.float32)
        s = np.asarray(s0, np.float32)
        return ((a * s + b) < one).astype(np.float32)

    cmp_xy = reg(
        "ANTK_CMP_XY",
        Spec(body=(sq(Src0) + sq(Src1)) < One, reference=_ref_cmp_xy),
    )
    cmp_z = reg(
        "ANTK_CMP_Z",
        Spec(body=(Src0 * C0 + Src1) < One, reference=_ref_cmp_z),
    )
    return cmp_xy, cmp_z


def _build_program():
    import concourse.bass as bass
    import concourse.tile as tile
    from concourse import mybir

    f32 = mybir.dt.float32
    u8 = mybir.dt.uint8
    Alu = mybir.AluOpType
    Act = mybir.ActivationFunctionType

    cmp_xy, cmp_z = _register_ops()

    nc = bass.Bass()

    # pc[p, w, c]: partition p = b*32+s owns slab s of batch b (a plain
    # reshape of the per-core [4, 131072, 6] input).
    pc = nc.declare_dram_parameter("pc", [P, W, C], f32, isOutput=False)
    # tt[b, k]: flattened 4x4 transform of batch b.
    tt = nc.declare_dram_parameter("tt", [BPC, 16], f32, isOutput=False)
    # m[p, 0, w] = (p_z < 1), m[p, 1, w] = (p_x^2 + p_y^2 < 1), both u8 0/1.
    m = nc.declare_dram_parameter("m", [P, 2, W], u8, isOutput=True)

    with tile.TileContext(nc) as tc:
        with (
            tc.tile_pool(name="singles", bufs=1) as singles,
            tc.tile_pool(name="data", bufs=3) as dpool,
            tc.tile_pool(name="work", bufs=3) as wpool,
            tc.tile_pool(name="mask", bufs=3) as mpool,
        ):
            # ttb[p, k] = tt[p//32, k]: per-partition transform scalars.
            ttb = singles.tile([P, 16], f32)
            tt_flat = tt[:].rearrange("b k -> (b k)")

            # all input DMAs issued up front on SP (buffer-reuse waits are
            # inserted automatically by the tile framework). The first data
            # chunk goes ahead of ttb: ttb is only needed by the b_e ops,
            # while chunk 0's copies gate the whole pipeline ramp.
            CWMAX = max(cw for _, cw in CHUNKS)
            CCMAX = max(cw for _, cw, _g in CCHUNKS)
            datas = []
            for ci, (c0_, cw) in enumerate(CHUNKS):
                data = dpool.tile(
                    [P, CWMAX, C], f32, tag="data", name=f"data{ci}"
                )
                nc.sync.dma_start(
                    out=data[:, :cw, :], in_=pc[:, c0_ : c0_ + cw, :]
                )
                datas.append(data)
                if ci == 0:
                    nc.sync.dma_start(
                        out=ttb[:],
                        in_=bass.AP(
                            tensor=tt_flat.tensor,
                            offset=tt_flat.offset,
                            ap=[[16, BPC], [0, SLABS], [1, 16]],
                        ),
                    )

            # warm the ACT function table early (overlaps the first loads)
            warm = singles.tile([P, 1], f32)
            nc.scalar.activation(out=warm[:], in_=ttb[:, 0:1], func=Act.Identity)

            def rotc(d, e):
                k = 4 * d + e
                return ttb[:, k : k + 1]

            def trn(e):
                k = 4 * e + 3
                return ttb[:, k : k + 1]

            mask_outs = []  # (c0_, cw, mtile): all issued on SP at the end

            for ci, (c0_, cw, grains) in enumerate(CCHUNKS):
                bt = [
                    wpool.tile([P, CCMAX], f32, tag=f"bt{e}", name=f"bt{e}_{ci}")[:, :cw]
                    for e in range(3)
                ]
                cp01 = wpool.tile(
                    [P, CCMAX, 2], f32, tag="cp01", name=f"cp01_{ci}"
                )[:, :cw, :]
                zt = wpool.tile([P, CCMAX], f32, tag="zt", name=f"zt_{ci}")[:, :cw]
                # per-grain deinterleave copies into slices of the wide tiles
                lo = 0
                for gi in grains:
                    g0, gw = CHUNKS[gi]
                    data = datas[gi]
                    nc.scalar.activation(
                        out=cp01[:, lo : lo + gw, :], in_=data[:, :gw, 0:2],
                        func=Act.Identity,
                    )
                    nc.scalar.activation(
                        out=zt[:, lo : lo + gw], in_=data[:, :gw, 2],
                        func=Act.Identity,
                    )
                    lo += gw
                for e in range(3):
                    nc.scalar.activation(
                        out=bt[e], in_=zt, func=Act.Identity,
                        bias=trn(e), scale=rotc(2, e),
                    )
                xs = cp01[:, :, 0]
                ys = cp01[:, :, 1]

                # ---- DVE: u_e = y*r1e + b_e, p_e = x*r0e + u_e ----
                for e in range(3):
                    nc.vector.scalar_tensor_tensor(
                        out=bt[e], in0=ys, scalar=rotc(1, e), in1=bt[e],
                        op0=Alu.mult, op1=Alu.add,
                    )
                for e in range(2):
                    nc.vector.scalar_tensor_tensor(
                        out=bt[e], in0=xs, scalar=rotc(0, e), in1=bt[e],
                        op0=Alu.mult, op1=Alu.add,
                    )

                # ---- fused masks ----
                mt = mpool.tile(
                    [P, 2, CCMAX], u8, tag="mt", name=f"mt{ci}"
                )[:, :, :cw]
                nc.vector._custom_dve(
                    cmp_z, out=mt[:, 0, :], in0=xs, in1=bt[2], s0=rotc(0, 2)
                )
                nc.vector._custom_dve(
                    cmp_xy, out=mt[:, 1, :], in0=bt[0], in1=bt[1]
                )
                mask_outs.append((c0_, cw, mt))

            # mask-out DMAs on SP, after every input DMA is already queued —
            # SP is otherwise idle, so mask waits never block anything.
            for c0p, cwp, mtp in mask_outs:
                nc.sync.dma_start(out=m[:, :, c0p : c0p + cwp], in_=mtp)

    # Populate .instr bytes for InstISA subclasses (InstCustomDveAnt) —
    # raw Bass skips this pass; without it the NEFF compiler sees empty
    # .instr and fails with "ISA wrong length".
    mybir.codegen_inst_isa_subclasses(nc)
    if SPILL_WAITS:
        _split_excess_waits(nc)
    nc.finalize()
    return nc


def _get_program():
    if "nc" not in _CACHE:
        _CACHE["nc"] = _build_program()
    return _CACHE["nc"]


def make_in_maps(pointclouds, tt16):
    """Per-core input dicts. pointclouds [B,N,C] f32, tt16 [B,16] f32."""
    in_maps = []
    for c in range(NCORES):
        sl = slice(c * BPC, (c + 1) * BPC)
        in_maps.append(
            {
                "pc": np.ascontiguousarray(pointclouds[sl]).reshape(P, W, C),
                "tt": np.ascontiguousarray(tt16[sl]),
            }
        )
    return in_maps


def postprocess(results, pointclouds):
    """Combine the device geometric mask bytes with the (bit-exact, numpy
    f32) padded-row check, then stable-compact valid rows to the front with
    a zero tail. results[c]["m0"/"m1"] is [P, W] u8."""
    out = np.zeros((B, N, C), dtype=np.float32)
    for c in range(NCORES):
        mm = np.asarray(results[c]["m"])  # [P, 2, W]
        g0 = mm[:, 0, :].reshape(BPC, N)
        g1 = mm[:, 1, :].reshape(BPC, N)
        geo = (g0 == 1) & (g1 == 1)
        for b in range(BPC):
            gb = c * BPC + b
            nrm = pointclouds[gb, :, 3:]
            nsum = (nrm[:, 0] + nrm[:, 1]) + nrm[:, 2]  # matches jnp.sum order
            m = geo[b] & (nsum != 0)
            kk = int(m.sum())
            out[gb, :kk] = pointclouds[gb][m]
    return out


def kernel(pointclouds: np.ndarray, task_transform: np.ndarray) -> np.ndarray:
    from concourse.bass_utils import run_bass_kernel_spmd

    pointclouds = np.ascontiguousarray(pointclouds, dtype=np.float32)
    task_transform = np.ascontiguousarray(task_transform, dtype=np.float32)
    assert pointclouds.shape == (B, N, C), pointclouds.shape
    assert task_transform.shape == (B, 4, 4), task_transform.shape

    nc = _get_program()
    in_maps = make_in_maps(pointclouds, task_transform.reshape(B, 16))
    res = run_bass_kernel_spmd(nc, in_maps, core_ids=list(range(NCORES)))
    return postprocess(res.results, pointclouds)


# revision 33
# speedup vs baseline: 1.0522x; 1.0522x over previous
"""Trainium2 Bass kernel for nn_BaseNet_72533407694985.

Computes, per batch b:
  p = pts @ rot_b + trans_b            (pts = pointclouds[b,:, :3])
  valid = (p_x^2+p_y^2 < 1) & (p_z < 1) & (sum(normals) != 0)
  out[b] = stable-compact rows of pointclouds[b] where valid, zero tail.

Strategy (4 batches per core on 8 cores, unified 128-partition tiles):
  - Partition p = (batch b = p//32, slab s = p%32); each partition owns the
    contiguous 4096-point slab [s*4096, (s+1)*4096) of its batch, so every
    op covers all 4 batches at once with per-partition transform scalars.
  - Per column-chunk: ACT deinterleaves the point data (cp01 = (x,y) pairs,
    zt = z; stride-24B reads halve DVE throughput, so the copies are paid
    once on the otherwise-idle ACT) and computes b_e = z*r2e + t_e
    (scale/bias); DVE runs the six exact-chain fmas (stt) plus two fused
    custom DVE ops (registered at build time in dve_ops.OPS):
      CMP_Z  : m0 = (x*r02 + u2) < 1         (mult,add,is_lt — 3 stages)
      CMP_XY : m1 = (p0*p0 + p1*p1) < 1      (sq,sq,add,is_lt — 4 stages)
    writing the two geometric mask bytes directly (u8 0/1). Mask-out DMAs
    ride the otherwise-idle SP engine so their waits never stall compute.
  - Host applies the (bit-exact in numpy f32) padded-row check nsum != 0,
    ANDs the mask bytes, and does the stable compaction — same split the
    previous version of this kernel family used.
  - Arithmetic association kept bit-identical to the reference chain
    (z*r2e+t_e via ACT scale/bias, += y*r1e, += x*r0e via stt; squares,
    sum and compares as exact f32 mult/add/is_lt on the DVE ALUs).
"""

import numpy as np

B = 32
N = 131072
C = 6
P = 128
NCORES = 8
BPC = B // NCORES   # batches per core
SLABS = 32          # slabs per batch; BPC*SLABS = 128 partitions
W = N // SLABS      # 4096 points per partition

# (col0, width) chunks over the 4096-point slab: small chunks up front for
# a fast ramp, then a monotone taper [1024, 896, 768, 640]. The taper has
# the same chunk/op count as [1024,1024,1024,256] (so no cost when the run
# is DVE-bound) but shortens the serial delivery->copy->compute trail of
# the final grains when HBM bandwidth is degraded and T is delivery-bound
# (measured ~0.6-1.7us across taper steps at 38-41us stream spans).
CHUNKS = [(0, 256), (256, 512), (768, 1024), (1792, 896), (2688, 768), (3456, 640)]
# compute chunks: (col0, width, [grain indices]) — 1:1 with the DMA grains
# (wider fused compute chunks measured slower: they defer compute behind the
# later grain's delivery, which outweighs the saved per-op fixed cost)
CCHUNKS = [(c0, cw, [i]) for i, (c0, cw) in enumerate(CHUNKS)]

_CACHE = {}
SPILL_WAITS = True


def _split_excess_waits(nc):
    """Walrus codegen caps sync waits at 1 per instruction (2 for
    EventSemaphore). Spill extra waits into sem-only EventSemaphore nops
    inserted just before the overloaded instruction on the same engine."""
    from concourse import mybir

    n_spilled = 0
    for f in nc.m.functions:
        for blk in f.blocks:
            out = []
            changed = False
            for ins in blk.instructions:
                si = ins.sync_info
                cap = 2 if isinstance(ins, mybir.InstEventSemaphore) else 1
                if si is not None and len(si.on_wait) > cap:
                    waits = list(si.on_wait)
                    keep, spill = waits[:cap], waits[cap:]
                    k = 0
                    while spill:
                        chunk, spill = spill[:2], spill[2:]
                        out.append(
                            mybir.InstEventSemaphore(
                                name=f"{ins.name}_w{k}",
                                engine=ins.engine,
                                ins=[],
                                outs=[],
                                sync_info=mybir.SyncInfo(
                                    on_wait=chunk, on_update=[]
                                ),
                            )
                        )
                        k += 1
                        n_spilled += 1
                    si.on_wait = keep
                    changed = True
                out.append(ins)
            if changed:
                blk.instructions = out
    return n_spilled


def _register_ops():
    """Register the two fused mask ops in the custom-DVE registry (the
    documented extension point: append a DveOp to dve_ops.OPS). Idempotent."""
    import concourse.dve_ops as dvo
    from concourse.dve_spec import Spec, Src0, Src1, C0, One, sq, lower
    from concourse.dve_spec import _has_src1
    from concourse.dve_uop import DveOpSpec

    def reg(name, spec):
        for o in dvo.OPS:
            if o.name == name:
                return o
        row = dvo._CUSTOM_DVE_ROW_BASE + len(dvo.OPS)
        assert row < 0x20, "custom DVE opcode rows exhausted"
        dvo._SUB_OPCODE_FOR_NAME[name] = row
        sha = DveOpSpec(
            name=name,
            opcode=row,
            uops=lower(spec, ver="v3"),
            rd1_en=_has_src1(spec),
        ).sha("v3")
        op = dvo.DveOp(name, spec, subdim=False, uops_sha={"v3": sha})
        dvo.OPS.append(op)
        dvo.CUSTOM_DVE_SPECS[name] = spec
        return op

    one = np.float32(1.0)

    def _ref_cmp_xy(in0, in1, s0, s1, imm2):
        a = np.asarray(in0, np.float32)
        b = np.asarray(in1, np.float32)
        return ((a * a + b * b) < one).astype(np.float32)

    def _ref_cmp_z(in0, in1, s0, s1, imm2):
        a = np.asarray(in0, np.float32)
        b = np.asarray(in1, np.float32)
        s = np.asarray(s0, np.float32)
        return ((a * s + b) < one).astype(np.float32)

    cmp_xy = reg(
        "ANTK_CMP_XY",
        Spec(body=(sq(Src0) + sq(Src1)) < One, reference=_ref_cmp_xy),
    )
    cmp_z = reg(
        "ANTK_CMP_Z",
        Spec(body=(Src0 * C0 + Src1) < One, reference=_ref_cmp_z),
    )
    return cmp_xy, cmp_z


def _build_program():
    import concourse.bass as bass
    import concourse.tile as tile
    from concourse import mybir

    f32 = mybir.dt.float32
    u8 = mybir.dt.uint8
    Alu = mybir.AluOpType
    Act = mybir.ActivationFunctionType

    cmp_xy, cmp_z = _register_ops()

    nc = bass.Bass()

    # pc[p, w, c]: partition p = b*32+s owns slab s of batch b (a plain
    # reshape of the per-core [4, 131072, 6] input).
    pc = nc.declare_dram_parameter("pc", [P, W, C], f32, isOutput=False)
    # tt[b, k]: flattened 4x4 transform of batch b.
    tt = nc.declare_dram_parameter("tt", [BPC, 16], f32, isOutput=False)
    # m[p, 0, w] = (p_z < 1), m[p, 1, w] = (p_x^2 + p_y^2 < 1), both u8 0/1.
    m = nc.declare_dram_parameter("m", [P, 2, W], u8, isOutput=True)

    with tile.TileContext(nc) as tc:
        with (
            tc.tile_pool(name="singles", bufs=1) as singles,
            tc.tile_pool(name="data", bufs=3) as dpool,
            tc.tile_pool(name="work", bufs=3) as wpool,
            tc.tile_pool(name="mask", bufs=3) as mpool,
        ):
            # ttb[p, k] = tt[p//32, k]: per-partition transform scalars.
            ttb = singles.tile([P, 16], f32)
            tt_flat = tt[:].rearrange("b k -> (b k)")

            # all input DMAs issued up front on SP (buffer-reuse waits are
            # inserted automatically by the tile framework). The first data
            # chunk goes ahead of ttb: ttb is only needed by the b_e ops,
            # while chunk 0's copies gate the whole pipeline ramp.
            CWMAX = max(cw for _, cw in CHUNKS)
            CCMAX = max(cw for _, cw, _g in CCHUNKS)
            datas = []
            for ci, (c0_, cw) in enumerate(CHUNKS):
                data = dpool.tile(
                    [P, CWMAX, C], f32, tag="data", name=f"data{ci}"
                )
                nc.sync.dma_start(
                    out=data[:, :cw, :], in_=pc[:, c0_ : c0_ + cw, :]
                )
                datas.append(data)
                if ci == 0:
                    nc.sync.dma_start(
                        out=ttb[:],
                        in_=bass.AP(
                            tensor=tt_flat.tensor,
                            offset=tt_flat.offset,
                            ap=[[16, BPC], [0, SLABS], [1, 16]],
                        ),
                    )

            # warm the ACT function table early (overlaps the first loads)
            warm = singles.tile([P, 1], f32)
            nc.scalar.activation(out=warm[:], in_=ttb[:, 0:1], func=Act.Identity)

            def rotc(d, e):
                k = 4 * d + e
                return ttb[:, k : k + 1]

            def trn(e):
                k = 4 * e + 3
                return ttb[:, k : k + 1]

            mask_outs = []  # (c0_, cw, mtile): all issued on SP at the end

            for ci, (c0_, cw, grains) in enumerate(CCHUNKS):
                bt = [
                    wpool.tile([P, CCMAX], f32, tag=f"bt{e}", name=f"bt{e}_{ci}")[:, :cw]
                    for e in range(3)
                ]
                cp01 = wpool.tile(
                    [P, CCMAX, 2], f32, tag="cp01", name=f"cp01_{ci}"
                )[:, :cw, :]
                zt = wpool.tile([P, CCMAX], f32, tag="zt", name=f"zt_{ci}")[:, :cw]
                # per-grain deinterleave copies into slices of the wide tiles
                lo = 0
                for gi in grains:
                    g0, gw = CHUNKS[gi]
                    data = datas[gi]
                    nc.scalar.activation(
                        out=cp01[:, lo : lo + gw, :], in_=data[:, :gw, 0:2],
                        func=Act.Identity,
                    )
                    nc.scalar.activation(
                        out=zt[:, lo : lo + gw], in_=data[:, :gw, 2],
                        func=Act.Identity,
                    )
                    lo += gw
                for e in range(3):
                    nc.scalar.activation(
                        out=bt[e], in_=zt, func=Act.Identity,
                        bias=trn(e), scale=rotc(2, e),
                    )
                xs = cp01[:, :, 0]
                ys = cp01[:, :, 1]

                # ---- DVE: u_e = y*r1e + b_e, p_e = x*r0e + u_e ----
                for e in range(3):
                    nc.vector.scalar_tensor_tensor(
                        out=bt[e], in0=ys, scalar=rotc(1, e), in1=bt[e],
                        op0=Alu.mult, op1=Alu.add,
                    )
                for e in range(2):
                    nc.vector.scalar_tensor_tensor(
                        out=bt[e], in0=xs, scalar=rotc(0, e), in1=bt[e],
                        op0=Alu.mult, op1=Alu.add,
                    )

                # ---- fused masks ----
                mt = mpool.tile(
                    [P, 2, CCMAX], u8, tag="mt", name=f"mt{ci}"
                )[:, :, :cw]
                nc.vector._custom_dve(
                    cmp_z, out=mt[:, 0, :], in0=xs, in1=bt[2], s0=rotc(0, 2)
                )
                nc.vector._custom_dve(
                    cmp_xy, out=mt[:, 1, :], in0=bt[0], in1=bt[1]
                )
                mask_outs.append((c0_, cw, mt))

            # mask-out DMAs on SP, after every input DMA is already queued —
            # SP is otherwise idle, so mask waits never block anything.
            for c0p, cwp, mtp in mask_outs:
                nc.sync.dma_start(out=m[:, :, c0p : c0p + cwp], in_=mtp)

    # Populate .instr bytes for InstISA subclasses (InstCustomDveAnt) —
    # raw Bass skips this pass; without it the NEFF compiler sees empty
    # .instr and fails with "ISA wrong length".
    mybir.codegen_inst_isa_subclasses(nc)
    if SPILL_WAITS:
        _split_excess_waits(nc)
    nc.finalize()
    return nc


def _get_program():
    if "nc" not in _CACHE:
        _CACHE["nc"] = _build_program()
    return _CACHE["nc"]


def make_in_maps(pointclouds, tt16):
    """Per-core input dicts. pointclouds [B,N,C] f32, tt16 [B,16] f32."""
    in_maps = []
    for c in range(NCORES):
        sl = slice(c * BPC, (c + 1) * BPC)
        in_maps.append(
            {
                "pc": np.ascontiguousarray(pointclouds[sl]).reshape(P, W, C),
                "tt": np.ascontiguousarray(tt16[sl]),
            }
        )
    return in_maps


def postprocess(results, pointclouds):
    """Combine the device geometric mask bytes with the (bit-exact, numpy
    f32) padded-row check, then stable-compact valid rows to the front with
    a zero tail. results[c]["m0"/"m1"] is [P, W] u8."""
    out = np.zeros((B, N, C), dtype=np.float32)
    for c in range(NCORES):
        mm = np.asarray(results[c]["m"])  # [P, 2, W]
        g0 = mm[:, 0, :].reshape(BPC, N)
        g1 = mm[:, 1, :].reshape(BPC, N)
        geo = (g0 == 1) & (g1 == 1)
        for b in range(BPC):
            gb = c * BPC + b
            nrm = pointclouds[gb, :, 3:]
            nsum = (nrm[:, 0] + nrm[:, 1]) + nrm[:, 2]  # matches jnp.sum order
            m = geo[b] & (nsum != 0)
            kk = int(m.sum())
            out[gb, :kk] = pointclouds[gb][m]
    return out


def kernel(pointclouds: np.ndarray, task_transform: np.ndarray) -> np.ndarray:
    from concourse.bass_utils import run_bass_kernel_spmd

    pointclouds = np.ascontiguousarray(pointclouds, dtype=np.float32)
    task_transform = np.ascontiguousarray(task_transform, dtype=np.float32)
    assert pointclouds.shape == (B, N, C), pointclouds.shape
    assert task_transform.shape == (B, 4, 4), task_transform.shape

    nc = _get_program()
    in_maps = make_in_maps(pointclouds, task_transform.reshape(B, 16))
    res = run_bass_kernel_spmd(nc, in_maps, core_ids=list(range(NCORES)))
    return postprocess(res.results, pointclouds)
